# revision 8
# baseline (speedup 1.0000x reference)
"""Trainium2 Bass kernel for a 16-head causal attention layer with q/k RMSNorm.

Full-problem shapes: x [4, 2048, 2048], Wq/Wk/Wv [2048, 2048], Wo [2048, 2048],
16 heads x head_dim 128.

Sharding over 8 NeuronCores: core c = 2*b + g handles batch b (of 4) and head
group g (of 2, 8 heads each).  Each core computes its 8 heads' attention output
and the partial output projection restricted to its head-group's columns of Wo;
the host sums the two partials per batch.

Everything on-core lives in a transposed-friendly layout:
  - host supplies xT = x[b].T (bf16), WqT/WkT/WvT = W[g-rows].T (bf16),
    WoT = Wo[:, g-cols].T (bf16)
  - q/k are computed in natural [t, j] layout (so RMSNorm reduces along the
    free axis), normalized, then PE-transposed per head into qnT/knT [hd, t]
  - scores are computed transposed, ST[j_key, i_query]; the softmax needs no
    max-subtraction because RMSNorm bounds |q.k|/sqrt(hd) by sqrt(128)~11.3
  - the denominator D[i] = colsum(exp) comes from a ones-vector matmul on the
    PE; 1/D is broadcast across partitions with a K=1 fp32r matmul
  - PV and the output projection both consume/produce the transposed layout,
    so the core writes outT [e, t]; the host transposes back and sums pairs.
"""

import numpy as np
import ml_dtypes

# ---- problem constants (hardcoded; kernel.py must be self-contained) ----
B = 4
T = 2048
D_MODEL = 2048
N_HEADS = 16
HD = 128
EPS = 1e-5
N_CORES = 8

H = 8                 # heads per core
JW = H * HD           # 1024, per-core projection width
P = 128               # partitions
IB = 512              # query block width (one PSUM bank of fp32)
NT = T // P           # 16 t-tiles
ND = D_MODEL // P     # 16 contraction tiles
NE = D_MODEL // P     # 16 output-dim tiles
NJB = JW // IB        # 2 j-blocks per projection
NIB = T // IB         # 4 query blocks
SCALE = HD ** -0.5

_CACHE = {}


def build_bass():
    import concourse.bacc as bacc
    import concourse.mybir as mybir
    import concourse.tile as tile
    from concourse import masks

    dt = mybir.dt
    f32 = dt.float32
    bf16 = dt.bfloat16
    f32r = dt.float32r
    AF = mybir.ActivationFunctionType
    ALU = mybir.AluOpType

    nc = bacc.Bacc("TRN2", target_bir_lowering=False, debug=False,
                   num_devices=N_CORES)

    xT_d = nc.dram_tensor("xT", [D_MODEL, T], bf16, kind="ExternalInput")
    wqT_d = nc.dram_tensor("wqT", [D_MODEL, JW], bf16, kind="ExternalInput")
    wkT_d = nc.dram_tensor("wkT", [D_MODEL, JW], bf16, kind="ExternalInput")
    wvT_d = nc.dram_tensor("wvT", [D_MODEL, JW], bf16, kind="ExternalInput")
    woT_d = nc.dram_tensor("woT", [JW, D_MODEL], bf16, kind="ExternalInput")
    gq_d = nc.dram_tensor("gq", [HD, 1], f32, kind="ExternalInput")
    gk_d = nc.dram_tensor("gk", [HD, 1], f32, kind="ExternalInput")
    outT_d = nc.dram_tensor("outT", [D_MODEL, T], f32, kind="ExternalOutput")

    xT_v = xT_d.ap().rearrange("(dn p) t -> dn p t", p=P)
    wqT_v = wqT_d.ap().rearrange("(dn p) j -> dn p j", p=P)
    wkT_v = wkT_d.ap().rearrange("(dn p) j -> dn p j", p=P)
    wvT_v = wvT_d.ap().rearrange("(dn p) j -> dn p j", p=P)
    woT_v = woT_d.ap().rearrange("(jh p) e -> jh p e", p=P)
    outT_v = outT_d.ap().rearrange("(en p) t -> en p t", p=P)

    with tile.TileContext(nc) as tc:
        from contextlib import ExitStack
        with ExitStack() as top:
            const = top.enter_context(tc.tile_pool(name="const", bufs=1))
            idn = const.tile([P, P], bf16, tag="idn")
            masks.make_identity(nc, idn[:])
            ones_col = const.tile([P, 1], bf16, tag="ones_col")
            nc.gpsimd.memset(ones_col[:], 1.0)
            ones_row = const.tile([1, P], f32r, tag="ones_row")
            nc.gpsimd.memset(ones_row[:].bitcast(f32), 1.0)
            gq_sb = const.tile([P, 1], f32, tag="gq")
            nc.sync.dma_start(gq_sb[:], gq_d.ap())
            gk_sb = const.tile([P, 1], f32, tag="gk")
            nc.sync.dma_start(gk_sb[:], gk_d.ap())
            epsb = const.tile([P, 1], f32, tag="epsb")
            nc.gpsimd.memset(epsb[:], EPS)
            # additive causal masks for the 4 diagonal [j=128, i=512] blocks:
            # keep iff ii - jj - 128*jtd >= 0
            cmask = []
            for jtd in range(IB // P):
                m = const.tile([P, IB], f32, tag=f"cmask{jtd}")
                nc.gpsimd.memset(m[:], 0.0)
                nc.gpsimd.affine_select(
                    out=m[:], in_=m[:], compare_op=ALU.is_ge,
                    fill=-1e30, base=-P * jtd, pattern=[[1, IB]],
                    channel_multiplier=-1,
                )
                cmask.append(m)

            qk_persist = top.enter_context(tc.tile_pool(name="qk", bufs=1))
            qnT = [qk_persist.tile([P, T], bf16, tag=f"qnT{h}", name=f"qnT{h}") for h in range(H)]
            knT = [qk_persist.tile([P, T], bf16, tag=f"knT{h}", name=f"knT{h}") for h in range(H)]

            # ---------------- phases Q and K (xT resident) ----------------
            with ExitStack() as ph:
                xpool = ph.enter_context(tc.tile_pool(name="xT", bufs=1))
                x_sb = [xpool.tile([P, T], bf16, tag=f"x{dn}", name=f"x{dn}") for dn in range(ND)]
                for dn in range(ND):
                    nc.sync.dma_start(x_sb[dn][:], xT_v[dn])

                for w_view, dstT, g_sb in ((wqT_v, qnT, gq_sb),
                                           (wkT_v, knT, gk_sb)):
                    with ExitStack() as pp:
                        wpool = pp.enter_context(tc.tile_pool(name="w", bufs=1))
                        work = pp.enter_context(tc.tile_pool(name="wk", bufs=3))
                        psp = pp.enter_context(
                            tc.tile_pool(name="psp", bufs=3, space="PSUM"))
                        pst = pp.enter_context(
                            tc.tile_pool(name="pst", bufs=4, space="PSUM"))
                        w_sb = [wpool.tile([P, JW], bf16, tag=f"w{dn}", name=f"w{dn}")
                                for dn in range(ND)]
                        for dn in range(ND):
                            nc.sync.dma_start(w_sb[dn][:], w_view[dn])
                        for tn in range(NT):
                            for jb in range(NJB):
                                ps = psp.tile([P, IB], f32, tag="proj")
                                for dn in range(ND):
                                    nc.tensor.matmul(
                                        ps[:],
                                        x_sb[dn][:, tn * P:(tn + 1) * P],
                                        w_sb[dn][:, jb * IB:(jb + 1) * IB],
                                        start=(dn == 0), stop=(dn == ND - 1))
                                # per-head RMSNorm along free dim; ACT Square
                                # with accum_out gives the per-chunk row sums
                                sq = work.tile([P, IB], f32, tag="sq")
                                ssq = work.tile([P, 4], f32, tag="ssq")
                                for hc in range(4):
                                    nc.scalar.activation(
                                        sq[:, hc * HD:(hc + 1) * HD],
                                        ps[:, hc * HD:(hc + 1) * HD],
                                        AF.Square,
                                        accum_out=ssq[:, hc:hc + 1])
                                rms = work.tile([P, 4], f32, tag="rms")
                                nc.scalar.activation(rms[:], ssq[:], AF.Sqrt,
                                                     bias=epsb[:],
                                                     scale=1.0 / HD)
                                rinv = work.tile([P, 4], f32, tag="rinv")
                                nc.vector.reciprocal(rinv[:], rms[:])
                                qn = work.tile([P, IB], bf16, tag="qn")
                                for hc in range(4):
                                    nc.vector.tensor_scalar_mul(
                                        qn[:, hc * HD:(hc + 1) * HD],
                                        ps[:, hc * HD:(hc + 1) * HD],
                                        rinv[:, hc:hc + 1])
                                for hc in range(4):
                                    h = jb * 4 + hc
                                    tp = pst.tile([P, P], bf16, tag="tr")
                                    nc.tensor.transpose(
                                        tp[:], qn[:, hc * HD:(hc + 1) * HD],
                                        idn[:])
                                    nc.vector.tensor_scalar_mul(
                                        dstT[h][:, tn * P:(tn + 1) * P],
                                        tp[:], g_sb[:])

            # ---------------- phase V (xT re-streamed in two T-halves) --------
            v_pool = top.enter_context(tc.tile_pool(name="v", bufs=1))
            v_sb = [v_pool.tile([P, JW], bf16, tag=f"v{tn}", name=f"v{tn}") for tn in range(NT)]
            with ExitStack() as ph:
                wpool = ph.enter_context(tc.tile_pool(name="wv", bufs=1))
                xbpool = ph.enter_context(tc.tile_pool(name="xh", bufs=1))
                psp = ph.enter_context(
                    tc.tile_pool(name="psv", bufs=3, space="PSUM"))
                w_sb = [wpool.tile([P, JW], bf16, tag=f"w{dn}", name=f"wv{dn}")
                        for dn in range(ND)]
                for dn in range(ND):
                    nc.sync.dma_start(w_sb[dn][:], wvT_v[dn])
                TH = T // 2
                for half in range(2):
                    xh = [xbpool.tile([P, TH], bf16, tag=f"xh{dn}", name=f"xh{dn}")
                          for dn in range(ND)]
                    for dn in range(ND):
                        nc.sync.dma_start(
                            xh[dn][:], xT_v[dn][:, half * TH:(half + 1) * TH])
                    for tl in range(TH // P):
                        tn = half * (TH // P) + tl
                        for jb in range(NJB):
                            ps = psp.tile([P, IB], f32, tag="vproj")
                            for dn in range(ND):
                                nc.tensor.matmul(
                                    ps[:], xh[dn][:, tl * P:(tl + 1) * P],
                                    w_sb[dn][:, jb * IB:(jb + 1) * IB],
                                    start=(dn == 0), stop=(dn == ND - 1))
                            nc.vector.tensor_copy(
                                v_sb[tn][:, jb * IB:(jb + 1) * IB], ps[:])

            # ---------------- phase 2: attention + output projection ----------
            with ExitStack() as ph:
                wopool = ph.enter_context(tc.tile_pool(name="wo", bufs=1))
                wo_sb = [wopool.tile([P, D_MODEL], bf16, tag=f"wo{jh}", name=f"wo{jh}")
                         for jh in range(H)]
                for jh in range(H):
                    nc.sync.dma_start(wo_sb[jh][:], woT_v[jh])
                pexp_pool = ph.enter_context(tc.tile_pool(name="pexp", bufs=6))
                ot_pool = ph.enter_context(tc.tile_pool(name="ot", bufs=10))
                osb_pool = ph.enter_context(tc.tile_pool(name="osb", bufs=3))
                rd_pool = ph.enter_context(tc.tile_pool(name="rd", bufs=3))
                ps_st = ph.enter_context(
                    tc.tile_pool(name="ps_st", bufs=2, space="PSUM"))
                ps_d = ph.enter_context(
                    tc.tile_pool(name="ps_d", bufs=2, space="PSUM"))
                ps_ot = ph.enter_context(
                    tc.tile_pool(name="ps_ot", bufs=2, space="PSUM"))
                ps_op = ph.enter_context(
                    tc.tile_pool(name="ps_op", bufs=2, space="PSUM"))

                for c in range(NIB):
                    ots = []
                    for h in range(H):
                        qs = qnT[h][:, c * IB:(c + 1) * IB]
                        nj = (IB // P) * (c + 1)
                        pot = ps_ot.tile([P, IB], f32, tag="ot")
                        pd = ps_d.tile([1, IB], f32, tag="d")
                        for jt in range(nj):
                            st = ps_st.tile([P, IB], f32, tag="st")
                            nc.tensor.matmul(
                                st[:], knT[h][:, jt * P:(jt + 1) * P], qs,
                                start=True, stop=True)
                            jtd = jt - (IB // P) * c
                            if jtd >= 0:
                                nc.vector.tensor_tensor(
                                    st[:], st[:], cmask[jtd][:], op=ALU.add)
                            pe = pexp_pool.tile([P, IB], bf16, tag="pexp")
                            nc.scalar.activation(pe[:], st[:], AF.Exp,
                                                 scale=SCALE)
                            nc.tensor.matmul(pd[:], ones_col[:], pe[:],
                                             start=(jt == 0),
                                             stop=(jt == nj - 1))
                            nc.tensor.matmul(
                                pot[:], v_sb[jt][:, h * HD:(h + 1) * HD],
                                pe[:], start=(jt == 0), stop=(jt == nj - 1))
                        rd = rd_pool.tile([1, IB], f32r, tag="rd")
                        with nc.allow_low_precision(
                                reason="1/D broadcast via f32r matmul"):
                            nc.vector.reciprocal(rd[:], pd[:])
                        bc = ps_st.tile([P, IB], f32, tag="st")
                        nc.tensor.matmul(
                            bc[:], ones_row[:], rd[:],
                            start=True, stop=True)
                        bc_sb = rd_pool.tile([P, IB], f32, tag="bc_sb")
                        nc.vector.tensor_copy(bc_sb[:], bc[:])
                        ot = ot_pool.tile([P, IB], bf16, tag="ot_sb")
                        nc.vector.tensor_mul(ot[:], pot[:], bc_sb[:])
                        ots.append(ot)
                    for et in range(NE):
                        po = ps_op.tile([P, IB], f32, tag="op")
                        for h in range(H):
                            nc.tensor.matmul(
                                po[:], wo_sb[h][:, et * P:(et + 1) * P],
                                ots[h][:], start=(h == 0), stop=(h == H - 1))
                        osb = osb_pool.tile([P, IB], f32, tag="osb")
                        nc.vector.tensor_copy(osb[:], po[:])
                        nc.sync.dma_start(
                            outT_v[et][:, c * IB:(c + 1) * IB], osb[:])

    nc.compile()
    return nc


def shard_inputs(x, Wq, Wk, Wv, Wo, gq, gk):
    bf = ml_dtypes.bfloat16
    in_maps = []
    for c in range(N_CORES):
        b, g = divmod(c, 2)
        rows = slice(g * JW, (g + 1) * JW)
        in_maps.append({
            "xT": np.ascontiguousarray(x[b].T).astype(bf),
            "wqT": np.ascontiguousarray(Wq[rows].T).astype(bf),
            "wkT": np.ascontiguousarray(Wk[rows].T).astype(bf),
            "wvT": np.ascontiguousarray(Wv[rows].T).astype(bf),
            "woT": np.ascontiguousarray(Wo[:, rows].T).astype(bf),
            "gq": gq.reshape(HD, 1).astype(np.float32),
            "gk": gk.reshape(HD, 1).astype(np.float32),
        })
    return in_maps


def gather_outputs(results):
    out = np.empty((B, T, D_MODEL), dtype=np.float32)
    for b in range(B):
        acc = results[2 * b]["outT"] + results[2 * b + 1]["outT"]
        out[b] = acc.T
    return out


def kernel(x, Wq, Wk, Wv, Wo, gq, gk, _trace=False):
    from concourse.bass_utils import run_bass_kernel_spmd

    x = np.asarray(x, dtype=np.float32)
    Wq = np.asarray(Wq, dtype=np.float32)
    Wk = np.asarray(Wk, dtype=np.float32)
    Wv = np.asarray(Wv, dtype=np.float32)
    Wo = np.asarray(Wo, dtype=np.float32)
    gq = np.asarray(gq, dtype=np.float32)
    gk = np.asarray(gk, dtype=np.float32)

    if "nc" not in _CACHE:
        _CACHE["nc"] = build_bass()
    nc = _CACHE["nc"]

    in_maps = shard_inputs(x, Wq, Wk, Wv, Wo, gq, gk)
    res = run_bass_kernel_spmd(nc, in_maps, core_ids=list(range(N_CORES)),
                               trace=_trace)
    out = gather_outputs(res.results)
    if _trace:
        return out, res
    return out


if __name__ == "__main__":
    rng = np.random.default_rng(0)
    s = D_MODEL ** -0.5
    inputs = {
        "x": rng.standard_normal((B, T, D_MODEL), dtype=np.float32),
        "Wq": rng.standard_normal((D_MODEL, D_MODEL), dtype=np.float32) * s,
        "Wk": rng.standard_normal((D_MODEL, D_MODEL), dtype=np.float32) * s,
        "Wv": rng.standard_normal((D_MODEL, D_MODEL), dtype=np.float32) * s,
        "Wo": rng.standard_normal((D_MODEL, D_MODEL), dtype=np.float32) * s,
        "gq": np.ones(HD, np.float32),
        "gk": np.ones(HD, np.float32),
    }
    out = kernel(**inputs)
    print(out.shape, out.dtype)


# revision 9
# speedup vs baseline: 1.1936x; 1.1936x over previous
"""Trainium2 Bass kernel for a 16-head causal attention layer with q/k RMSNorm.

Full-problem shapes: x [4, 2048, 2048], Wq/Wk/Wv [2048, 2048], Wo [2048, 2048],
16 heads x head_dim 128.

Sharding over 8 NeuronCores: core c = 2*b + g handles batch b (of 4) and head
group g (of 2, 8 heads each).  Each core computes its 8 heads' attention output
and the partial output projection restricted to its head-group's columns of Wo;
the host sums the two partials per batch and transposes back.

Layout strategy (everything transposed, [feature, token]):
  - host supplies xT = x[b].T, WqT/WkT/WvT = W[g-rows].T, WoT = Wo[:, g-cols].T,
    all bf16
  - q/k are computed directly transposed per head, qT/kT [hd, t]: the weight
    tile is the stationary operand, xT the moving one
  - RMSNorm over hd (the partition dim) uses an all-ones [128,128] matmul of
    the squares, which yields the sum broadcast across all partitions; the
    normalize is then one scalar_tensor_tensor (x*g * rinv) on DVE
  - scores are computed transposed, ST[j_key, i_query]; softmax needs no
    max-subtraction because RMSNorm bounds |q.k|/sqrt(hd) by sqrt(128)~11.3
  - causal masking multiplies exp() by a 0/1 bf16 mask (diagonal blocks only)
  - the denominator D[i] = colsum(P~) comes from an all-ones [128,128] matmul,
    which lands already broadcast across partitions; normalization is a DVE
    reciprocal (PSUM->SBUF) + multiply
  - PV and the output projection both consume/produce the transposed layout,
    so the core writes outT [e, t] fp32.
"""

import numpy as np
import ml_dtypes

# ---- problem constants (hardcoded; kernel.py must be self-contained) ----
B = 4
T = 2048
D_MODEL = 2048
N_HEADS = 16
HD = 128
EPS = 1e-5
N_CORES = 8

H = 8                 # heads per core
JW = H * HD           # 1024, per-core projection width
P = 128               # partitions
IB = 512              # query block width (one PSUM bank of fp32)
NT = T // P           # 16 t-tiles
ND = D_MODEL // P     # 16 contraction tiles
NE = D_MODEL // P     # 16 output-dim tiles
NIB = T // IB         # 4 query blocks
NTB = T // IB         # 4 t-blocks in projections
SCALE = HD ** -0.5

_CACHE = {}


def build_bass():
    import concourse.bacc as bacc
    import concourse.mybir as mybir
    import concourse.tile as tile
    from contextlib import ExitStack

    dt = mybir.dt
    f32 = dt.float32
    bf16 = dt.bfloat16
    AF = mybir.ActivationFunctionType
    ALU = mybir.AluOpType

    nc = bacc.Bacc("TRN2", target_bir_lowering=False, debug=False,
                   num_devices=N_CORES)

    xT_d = nc.dram_tensor("xT", [D_MODEL, T], bf16, kind="ExternalInput")
    wqT_d = nc.dram_tensor("wqT", [D_MODEL, JW], bf16, kind="ExternalInput")
    wkT_d = nc.dram_tensor("wkT", [D_MODEL, JW], bf16, kind="ExternalInput")
    wvT_d = nc.dram_tensor("wvT", [D_MODEL, JW], bf16, kind="ExternalInput")
    woT_d = nc.dram_tensor("woT", [JW, D_MODEL], bf16, kind="ExternalInput")
    gq_d = nc.dram_tensor("gq", [HD, 1], f32, kind="ExternalInput")
    gk_d = nc.dram_tensor("gk", [HD, 1], f32, kind="ExternalInput")
    outT_d = nc.dram_tensor("outT", [D_MODEL, T], f32, kind="ExternalOutput")

    xT_v = xT_d.ap().rearrange("(dn p) t -> dn p t", p=P)
    wqT_v = wqT_d.ap().rearrange("(dn p) j -> dn p j", p=P)
    wkT_v = wkT_d.ap().rearrange("(dn p) j -> dn p j", p=P)
    wvT_v = wvT_d.ap().rearrange("(dn p) j -> dn p j", p=P)
    woT_v = woT_d.ap().rearrange("(jh p) e -> jh p e", p=P)
    outT_v = outT_d.ap().rearrange("(en p) t -> en p t", p=P)

    with tile.TileContext(nc) as tc:
        with ExitStack() as top:
            const = top.enter_context(tc.tile_pool(name="const", bufs=1))
            ones128 = const.tile([P, P], bf16, tag="ones128")
            nc.gpsimd.memset(ones128[:], 1.0)
            gq_sb = const.tile([P, 1], f32, tag="gq")
            nc.sync.dma_start(gq_sb[:], gq_d.ap())
            gk_sb = const.tile([P, 1], f32, tag="gk")
            nc.sync.dma_start(gk_sb[:], gk_d.ap())
            epsb = const.tile([P, 1], f32, tag="epsb")
            nc.gpsimd.memset(epsb[:], EPS)
            # multiplicative causal masks for the 4 diagonal [j=128, i=512]
            # blocks: keep (1) iff ii - jj - 128*jtd >= 0 else 0
            cmask = []
            for jtd in range(IB // P):
                m = const.tile([P, IB], bf16, tag=f"cmask{jtd}",
                               name=f"cmask{jtd}")
                nc.gpsimd.memset(m[:], 1.0)
                nc.gpsimd.affine_select(
                    out=m[:], in_=m[:], compare_op=ALU.is_ge,
                    fill=0.0, base=-P * jtd, pattern=[[1, IB]],
                    channel_multiplier=-1,
                )
                cmask.append(m)

            qk_persist = top.enter_context(tc.tile_pool(name="qk", bufs=1))
            qnT = [qk_persist.tile([P, T], bf16, tag=f"qnT{h}", name=f"qnT{h}")
                   for h in range(H)]
            knT = [qk_persist.tile([P, T], bf16, tag=f"knT{h}", name=f"knT{h}")
                   for h in range(H)]

            # xT stays resident for phases Q, K, V
            with ExitStack() as xctx:
                xpool = xctx.enter_context(tc.tile_pool(name="xT", bufs=1))
                x_sb = [xpool.tile([P, T], bf16, tag=f"x{dn}", name=f"x{dn}")
                        for dn in range(ND)]
                for dn in range(ND):
                    nc.sync.dma_start(x_sb[dn][:], xT_v[dn])

                # ---------- phases Q and K: qT/kT computed pre-transposed ----
                with ExitStack() as ph:
                    wqk = ph.enter_context(tc.tile_pool(name="wqk", bufs=2))
                    work = ph.enter_context(tc.tile_pool(name="wrk", bufs=3))
                    psq = ph.enter_context(
                        tc.tile_pool(name="psq", bufs=3, space="PSUM"))
                    pss = ph.enter_context(
                        tc.tile_pool(name="pss", bufs=2, space="PSUM"))
                    JQ = 256  # j-quarter round: 2 heads per W load round
                    for w_view, dstT, g_sb in ((wqT_v, qnT, gq_sb),
                                               (wkT_v, knT, gk_sb)):
                        for jq in range(JW // JQ):
                            w_sb = [wqk.tile([P, JQ], bf16, tag=f"w{dn}",
                                             name=f"w{dn}")
                                    for dn in range(ND)]
                            for dn in range(ND):
                                nc.sync.dma_start(
                                    w_sb[dn][:],
                                    w_view[dn][:, jq * JQ:(jq + 1) * JQ])
                            for jl in range(JQ // P):
                                h = jq * (JQ // P) + jl
                                for tb in range(NTB):
                                    ps = psq.tile([P, IB], f32, tag="qt")
                                    for dn in range(ND):
                                        nc.tensor.matmul(
                                            ps[:],
                                            w_sb[dn][:, jl * P:(jl + 1) * P],
                                            x_sb[dn][:, tb * IB:(tb + 1) * IB],
                                            start=(dn == 0),
                                            stop=(dn == ND - 1))
                                    sqt = work.tile([P, IB], bf16, tag="sqt")
                                    nc.scalar.activation(sqt[:], ps[:],
                                                         AF.Square)
                                    ssb = pss.tile([P, IB], f32, tag="ssb")
                                    nc.tensor.matmul(ssb[:], ones128[:],
                                                     sqt[:], start=True,
                                                     stop=True)
                                    rms = work.tile([P, IB], f32, tag="rms")
                                    nc.scalar.activation(rms[:], ssb[:],
                                                         AF.Sqrt,
                                                         bias=epsb[:],
                                                         scale=1.0 / HD)
                                    rinv = work.tile([P, IB], f32, tag="rinv")
                                    nc.vector.reciprocal(rinv[:], rms[:])
                                    nc.vector.scalar_tensor_tensor(
                                        out=dstT[h][:, tb * IB:(tb + 1) * IB],
                                        in0=ps[:], scalar=g_sb[:],
                                        in1=rinv[:],
                                        op0=ALU.mult, op1=ALU.mult)

                # ---------- phase V (natural layout; x stationary) ----------
                v_pool = xctx.enter_context(tc.tile_pool(name="v", bufs=1))
                v_sb = [v_pool.tile([P, JW], bf16, tag=f"v{tn}", name=f"v{tn}")
                        for tn in range(NT)]
                with ExitStack() as ph:
                    wv = ph.enter_context(tc.tile_pool(name="wv", bufs=1))
                    psv = ph.enter_context(
                        tc.tile_pool(name="psv", bufs=3, space="PSUM"))
                    for jb in range(JW // IB):
                        wv_sb = [wv.tile([P, IB], bf16, tag=f"wv{dn}",
                                         name=f"wv{dn}")
                                 for dn in range(ND)]
                        for dn in range(ND):
                            nc.sync.dma_start(
                                wv_sb[dn][:],
                                wvT_v[dn][:, jb * IB:(jb + 1) * IB])
                        for tn in range(NT):
                            ps = psv.tile([P, IB], f32, tag="vproj")
                            for dn in range(ND):
                                nc.tensor.matmul(
                                    ps[:], x_sb[dn][:, tn * P:(tn + 1) * P],
                                    wv_sb[dn][:],
                                    start=(dn == 0), stop=(dn == ND - 1))
                            nc.vector.tensor_copy(
                                v_sb[tn][:, jb * IB:(jb + 1) * IB], ps[:])

            # ---------- phase 2: attention + output projection --------------
            with ExitStack() as ph:
                wopool = ph.enter_context(tc.tile_pool(name="wo", bufs=1))
                wo_sb = [wopool.tile([P, D_MODEL], bf16, tag=f"wo{jh}",
                                     name=f"wo{jh}")
                         for jh in range(H)]
                for jh in range(H):
                    nc.sync.dma_start(wo_sb[jh][:], woT_v[jh])
                pexp_pool = ph.enter_context(tc.tile_pool(name="pexp", bufs=6))
                ot_pool = ph.enter_context(tc.tile_pool(name="ot", bufs=10))
                osb_pool = ph.enter_context(tc.tile_pool(name="osb", bufs=3))
                wrk2 = ph.enter_context(tc.tile_pool(name="wrk2", bufs=3))
                ps_st = ph.enter_context(
                    tc.tile_pool(name="ps_st", bufs=2, space="PSUM"))
                ps_d = ph.enter_context(
                    tc.tile_pool(name="ps_d", bufs=2, space="PSUM"))
                ps_ot = ph.enter_context(
                    tc.tile_pool(name="ps_ot", bufs=2, space="PSUM"))
                ps_op = ph.enter_context(
                    tc.tile_pool(name="ps_op", bufs=2, space="PSUM"))

                for c in range(NIB):
                    ots = []
                    for h in range(H):
                        qs = qnT[h][:, c * IB:(c + 1) * IB]
                        nj = (IB // P) * (c + 1)
                        pot = ps_ot.tile([P, IB], f32, tag="ot")
                        pd = ps_d.tile([P, IB], f32, tag="d")
                        for jt in range(nj):
                            st = ps_st.tile([P, IB], f32, tag="st")
                            nc.tensor.matmul(
                                st[:], knT[h][:, jt * P:(jt + 1) * P], qs,
                                start=True, stop=True)
                            pe = pexp_pool.tile([P, IB], bf16, tag="pexp")
                            nc.scalar.activation(pe[:], st[:], AF.Exp,
                                                 scale=SCALE)
                            jtd = jt - (IB // P) * c
                            if jtd >= 0:
                                nc.vector.tensor_mul(pe[:], pe[:],
                                                     cmask[jtd][:])
                            nc.tensor.matmul(pd[:], ones128[:], pe[:],
                                             start=(jt == 0),
                                             stop=(jt == nj - 1))
                            nc.tensor.matmul(
                                pot[:], v_sb[jt][:, h * HD:(h + 1) * HD],
                                pe[:], start=(jt == 0), stop=(jt == nj - 1))
                        rdb = wrk2.tile([P, IB], f32, tag="rdb")
                        nc.vector.reciprocal(rdb[:], pd[:])
                        ot = ot_pool.tile([P, IB], bf16, tag="ot_sb")
                        nc.vector.tensor_mul(ot[:], pot[:], rdb[:])
                        ots.append(ot)
                    for et in range(NE):
                        po = ps_op.tile([P, IB], f32, tag="op")
                        for h in range(H):
                            nc.tensor.matmul(
                                po[:], wo_sb[h][:, et * P:(et + 1) * P],
                                ots[h][:], start=(h == 0), stop=(h == H - 1))
                        osb = osb_pool.tile([P, IB], f32, tag="osb")
                        nc.vector.tensor_copy(osb[:], po[:])
                        nc.sync.dma_start(
                            outT_v[et][:, c * IB:(c + 1) * IB], osb[:])

    nc.compile()
    return nc


def shard_inputs(x, Wq, Wk, Wv, Wo, gq, gk):
    bf = ml_dtypes.bfloat16
    in_maps = []
    for c in range(N_CORES):
        b, g = divmod(c, 2)
        rows = slice(g * JW, (g + 1) * JW)
        in_maps.append({
            "xT": np.ascontiguousarray(x[b].T).astype(bf),
            "wqT": np.ascontiguousarray(Wq[rows].T).astype(bf),
            "wkT": np.ascontiguousarray(Wk[rows].T).astype(bf),
            "wvT": np.ascontiguousarray(Wv[rows].T).astype(bf),
            "woT": np.ascontiguousarray(Wo[:, rows].T).astype(bf),
            "gq": gq.reshape(HD, 1).astype(np.float32),
            "gk": gk.reshape(HD, 1).astype(np.float32),
        })
    return in_maps


def gather_outputs(results):
    out = np.empty((B, T, D_MODEL), dtype=np.float32)
    for b in range(B):
        acc = results[2 * b]["outT"] + results[2 * b + 1]["outT"]
        out[b] = acc.T
    return out


def kernel(x, Wq, Wk, Wv, Wo, gq, gk, _trace=False):
    from concourse.bass_utils import run_bass_kernel_spmd

    x = np.asarray(x, dtype=np.float32)
    Wq = np.asarray(Wq, dtype=np.float32)
    Wk = np.asarray(Wk, dtype=np.float32)
    Wv = np.asarray(Wv, dtype=np.float32)
    Wo = np.asarray(Wo, dtype=np.float32)
    gq = np.asarray(gq, dtype=np.float32)
    gk = np.asarray(gk, dtype=np.float32)

    if "nc" not in _CACHE:
        _CACHE["nc"] = build_bass()
    nc = _CACHE["nc"]

    in_maps = shard_inputs(x, Wq, Wk, Wv, Wo, gq, gk)
    res = run_bass_kernel_spmd(nc, in_maps, core_ids=list(range(N_CORES)),
                               trace=_trace)
    out = gather_outputs(res.results)
    if _trace:
        return out, res
    return out


if __name__ == "__main__":
    rng = np.random.default_rng(0)
    s = D_MODEL ** -0.5
    inputs = {
        "x": rng.standard_normal((B, T, D_MODEL), dtype=np.float32),
        "Wq": rng.standard_normal((D_MODEL, D_MODEL), dtype=np.float32) * s,
        "Wk": rng.standard_normal((D_MODEL, D_MODEL), dtype=np.float32) * s,
        "Wv": rng.standard_normal((D_MODEL, D_MODEL), dtype=np.float32) * s,
        "Wo": rng.standard_normal((D_MODEL, D_MODEL), dtype=np.float32) * s,
        "gq": np.ones(HD, np.float32),
        "gk": np.ones(HD, np.float32),
    }
    out = kernel(**inputs)
    print(out.shape, out.dtype)
